# revision 10
# baseline (speedup 1.0000x reference)
"""NsNet2 single-step (fc1 + 2x GRU cell + 3x FC) Trainium2 kernel.

Strategy:
  - Pure data parallel: batch B=32768 sharded as 4096 rows per NeuronCore (8 cores).
  - Feature-major ("transposed") layout on chip: activations live as [feat, batch]
    so every matmul's moving operand is already in [K, N] form -> zero on-chip
    transposes. Host transposes inputs/outputs (free; not on HW critical path).
  - bf16 matmuls (full PE rate) with fp32 PSUM accumulation; fp32 biases fused
    into ScalarE activation (sigmoid/tanh) or VectorE tensor_scalar (relu).
  - fc1 is folded into the GRU1 input-gate weights on the host (fc1 is linear and
    f1 is consumed only by GRU1's input matmuls):  (x@Wfc1.T+b) @ Wg.T =
    x @ (Wg@Wfc1).T + (Wg@b + bg).
  - z,r gates sum their input-side and hidden-side matmuls in one PSUM, so their
    contraction operands are K-concatenated ([x|h1] resp. [g1|h2]) on the host /
    on chip, saving ceil() waste: GRU1 zr K=657->6 chunks (vs 3+4), GRU2
    K=800->7 (vs 4+4).
  - Feature dims zero-padded to multiples of 128 where needed; padding never
    increases PE chunk count and keeps matmul contraction at 128 partitions.
"""

import os
import sys

import numpy as np
import ml_dtypes

sys.path.insert(0, "/opt/trn_rl_repo")

import concourse.bacc as bacc
import concourse.bass as bass
import concourse.mybir as mybir
import concourse.tile as tile
from concourse.bass import ts
from concourse.bass_utils import run_bass_kernel_spmd

BF16 = ml_dtypes.bfloat16

B, F, H, FF = 32768, 257, 400, 600
NCORES = 8
BPC = B // NCORES          # 4096 batch rows per core
Hp, FFp, Fp = 512, 640, 384  # padded feature dims
XH1 = 769                  # [x(257) | h1(400) | pad(112)] rows; 6 zr chunks + aligned h1 view at 257
ZR2K = 896                 # [g1(400) | h2(400) | pad(96)] -> 7 chunks
NB = 512                   # matmul free-dim tile (one PSUM bank of fp32)

AF = mybir.ActivationFunctionType
ALU = mybir.AluOpType

# packed bias column layout: name -> (offset, n_chunks)
BIAS_LAYOUT = {}
_off = 0
for _n, _c in (("bz1", 4), ("br1", 4), ("bnx1", 4), ("bnh1", 4),
               ("bz2", 4), ("br2", 4), ("bnx2", 4), ("bnh2", 4),
               ("bfc2", 5), ("bfc3", 5), ("bfc4", 3)):
    BIAS_LAYOUT[_n] = (_off, _c)
    _off += _c
BIAS_COLS = _off


def _pad2(a, rows, cols):
    out = np.zeros((rows, cols), dtype=np.float64)
    out[: a.shape[0], : a.shape[1]] = a
    return out


def _bias_tile(vec, padded):
    """Pack a [padded] bias vector as [128, padded//128] fp32 (partition-major)."""
    v = np.zeros(padded, dtype=np.float64)
    v[: vec.shape[0]] = vec
    return np.ascontiguousarray(v.reshape(padded // 128, 128).T).astype(np.float32)


def prepare_weights(inp):
    f64 = {k: np.asarray(v, dtype=np.float64) for k, v in inp.items()}
    w = {}

    # fc1 fold for GRU1 input side
    Wx = {}
    bx = {}
    for name in ("z", "r", "n"):
        Wx[name] = (f64[f"Wi{name}1"] @ f64["Wfc1"]).T          # [F, H]
        bx[name] = f64[f"bi{name}1"] + f64[f"Wi{name}1"] @ f64["bfc1"]

    # GRU1 z,r: K-concat [x(257) | h1(400)] -> rows 0..656 of XH1 space,
    # M = [z pad 512 | r pad 512] = 1024
    Wzr1 = np.zeros((768, 2 * Hp), dtype=np.float64)
    for g, name in enumerate(("z", "r")):
        Wzr1[:F, g * Hp : g * Hp + H] = Wx[name]
        Wzr1[F : F + H, g * Hp : g * Hp + H] = f64[f"Wh{name}1"].T
    w["Wzr1"] = Wzr1
    # GRU1 n input side: K = x chunks of XH1 (rows 0..383; rows 257+ are h1 -> zero)
    w["Wn1x"] = _pad2(Wx["n"], Fp, Hp)
    # GRU1 n hidden side: aligned h1 (XH1 rows 257..768)
    w["Wn1h"] = _pad2(f64["Whn1"].T, Hp, Hp)

    # GRU2 z,r: K-concat [g1(400) | h2(400)] -> 800 rows -> 7 chunks
    Wzr2 = np.zeros((ZR2K, 2 * Hp), dtype=np.float64)
    for g, name in enumerate(("z", "r")):
        Wzr2[:H, g * Hp : g * Hp + H] = f64[f"Wi{name}2"].T
        Wzr2[H : 2 * H, g * Hp : g * Hp + H] = f64[f"Wh{name}2"].T
    w["Wzr2"] = Wzr2
    # GRU2 n input side: K = g1 aligned (4 chunks; chunk 3 partitions 16.. are h2 -> zero)
    w["Wn2x"] = _pad2(f64["Win2"].T, Hp, Hp)
    w["Wn2h"] = _pad2(f64["Whn2"].T, Hp, Hp)

    w["Wfc2T"] = _pad2(f64["Wfc2"].T, Hp, FFp)    # [512, 640]
    w["Wfc3T"] = _pad2(f64["Wfc3"].T, FFp, FFp)   # [640, 640]
    w["Wfc4T"] = _pad2(f64["Wfc4"].T, FFp, Fp)    # [640, 384]

    weights = {k: np.ascontiguousarray(v).astype(BF16) for k, v in w.items()}

    parts = [
        ("bz1", _bias_tile(bx["z"] + f64["bhz1"], Hp)),
        ("br1", _bias_tile(bx["r"] + f64["bhr1"], Hp)),
        ("bnx1", _bias_tile(bx["n"], Hp)),
        ("bnh1", _bias_tile(f64["bhn1"], Hp)),
        ("bz2", _bias_tile(f64["biz2"] + f64["bhz2"], Hp)),
        ("br2", _bias_tile(f64["bir2"] + f64["bhr2"], Hp)),
        ("bnx2", _bias_tile(f64["bin2"], Hp)),
        ("bnh2", _bias_tile(f64["bhn2"], Hp)),
        ("bfc2", _bias_tile(f64["bfc2"], FFp)),
        ("bfc3", _bias_tile(f64["bfc3"], FFp)),
        ("bfc4", _bias_tile(f64["bfc4"], Fp)),
    ]
    biases = {"biasT": np.concatenate([p[1] for p in parts], axis=1)}
    return weights, biases


def build_nc(nbt=BPC, nb=NB):
    """Build the per-core Bass program. nbt = per-core batch, nb = free-dim tile."""
    nc = bacc.Bacc("TRN2", target_bir_lowering=False, debug=False)
    bf = mybir.dt.bfloat16
    f32 = mybir.dt.float32

    # xh1T rows: 0..256 = x.T, 257..656 = h1.T, 657..768 = zeros.
    # zr view = rows 0..767 (6 chunks); aligned-h1 view = rows 257..768 (4 chunks).
    xh1T = nc.declare_dram_parameter("xh1T", [XH1, nbt], bf, isOutput=False)
    # h2T rows 0..399 = h2.T, 400..511 zeros. Aligned view = 4 chunks; shifted
    # views for the [g1|h2] zr operand: rows 0..112 / 112..496.
    h2T = nc.declare_dram_parameter("h2T", [Hp, nbt], bf, isOutput=False)
    wd = {}
    for name, k, m in (
        ("Wzr1", 768, 2 * Hp), ("Wn1x", Fp, Hp), ("Wn1h", Hp, Hp),
        ("Wzr2", ZR2K, 2 * Hp), ("Wn2x", Hp, Hp), ("Wn2h", Hp, Hp),
        ("Wfc2T", Hp, FFp), ("Wfc3T", FFp, FFp), ("Wfc4T", FFp, Fp),
    ):
        wd[name] = nc.declare_dram_parameter(name, [k, m], bf, isOutput=False)
    biasT_d = nc.declare_dram_parameter("biasT", [128, BIAS_COLS], f32, isOutput=False)
    outT = nc.declare_dram_parameter("outT", [Fp, nbt], bf, isOutput=True)

    n_tiles = nbt // nb
    HC = Hp // 128  # 4 M-chunks per gate

    with tile.TileContext(nc) as tc:
        with (
            tc.tile_pool(name="wpool", bufs=1) as wpool,
            tc.tile_pool(name="bpool", bufs=1) as bpool,
            tc.tile_pool(name="io", bufs=2) as io,
            tc.tile_pool(name="inp", bufs=3) as inp,
            tc.tile_pool(name="act", bufs=3) as act,
            tc.tile_pool(name="psum", bufs=2, space="PSUM") as psum,
        ):
            # ACT-table warmup: first ScalarE transcendental carries the
            # ACT_TABLE_LOAD pseudo-inst; keep it off the critical chain.
            warm = bpool.tile([128, 1], f32, tag="warm")
            nc.vector.memset(warm, 0.0)
            nc.scalar.activation(warm, warm, AF.Sigmoid)

            W, BT = {}, {}
            _weng = [nc.scalar, nc.gpsimd]

            def load_w(name):
                dram = wd[name]
                k, m = dram.shape
                t = wpool.tile([128, k // 128, m], bf, tag=name)
                r = dram.rearrange("(c p) m -> p c m", p=128)
                for c in range(k // 128):
                    eng = _weng[(len(W) + c) % len(_weng)]
                    eng.dma_start(out=t[:, c, :], in_=r[:, c, :])
                W[name] = t

            biasT = bpool.tile([128, BIAS_COLS], f32, tag="biasT")
            nc.scalar.dma_start(out=biasT, in_=biasT_d[:, :])
            for _n, (_o, _c) in BIAS_LAYOUT.items():
                BT[_n] = biasT[:, _o : _o + _c]

            xh_zr = xh1T[0:768, :].rearrange("(c p) n -> p c n", p=128)
            h1_al = xh1T[257 : 257 + Hp, :].rearrange("(c p) n -> p c n", p=128)
            h2_al = h2T.rearrange("(c p) n -> p c n", p=128)
            h2_s0 = h2T[0:112, :]                     # -> partitions 16..127 of zr2 chunk 3
            h2_s1 = h2T[112:496, :].rearrange("(c p) n -> p c n", p=128)
            outT_r = outT.rearrange("(c p) n -> p c n", p=128)

            def load_inputs(t):
                sl = ts(t, nb)
                xh = inp.tile([128, 6, nb], bf, tag="xh")      # zr1 K operand
                for c in range(6):
                    nc.sync.dma_start(out=xh[:, c, :], in_=xh_zr[:, c, sl])
                h1s = inp.tile([128, HC, nb], bf, tag="h1s")   # aligned h1
                for c in range(HC):
                    nc.sync.dma_start(out=h1s[:, c, :], in_=h1_al[:, c, sl])
                h2s = inp.tile([128, HC, nb], bf, tag="h2s")   # aligned h2
                for c in range(HC):
                    nc.sync.dma_start(out=h2s[:, c, :], in_=h2_al[:, c, sl])
                h2p = inp.tile([128, 3, nb], bf, tag="h2p")    # zr2 chunks 4..6
                for c in range(3):
                    nc.sync.dma_start(out=h2p[:, c, :], in_=h2_s1[:, c, sl])
                return xh, h1s, h2s, h2p

            tile0_inputs = load_inputs(0)

            for name in ("Wzr1", "Wn1x", "Wn1h", "Wzr2", "Wn2x", "Wn2h",
                         "Wfc2T", "Wfc3T", "Wfc4T"):
                load_w(name)

            def matseq(ps, pairs):
                n = len(pairs)
                for i, (lhsT, rhs) in enumerate(pairs):
                    nc.tensor.matmul(ps, lhsT, rhs, start=(i == 0), stop=(i == n - 1))

            def gru(zr_ks, Wzr, nx_ks, Wnx, nh_ks, Wnh, h_al,
                    bz, br, bnx, bnh, out_chunk):
                """One GRU step. zr_ks: K-chunk APs for the z,r concat operand;
                nx_ks / nh_ks: K-chunks for the n gate's two sides; h_al: aligned
                hidden chunks (blend). out_chunk(m) -> output AP for chunk m."""
                for m in range(HC):
                    zr = []
                    for g, bias in ((0, bz), (1, br)):
                        col = g * Hp + m * 128
                        ps = psum.tile([128, nb], f32, tag="ps_zr")
                        matseq(ps, [(Wzr[:, k, col : col + 128], rhs)
                                    for k, rhs in enumerate(zr_ks)])
                        gate = act.tile([128, nb], bf, tag="gate")
                        nc.scalar.activation(gate, ps, AF.Sigmoid, bias=bias[:, m : m + 1])
                        zr.append(gate)
                    z, r = zr
                    col = m * 128
                    psx = psum.tile([128, nb], f32, tag="ps_nx")
                    matseq(psx, [(Wnx[:, k, col : col + 128], rhs)
                                 for k, rhs in enumerate(nx_ks)])
                    psh = psum.tile([128, nb], f32, tag="ps_nh")
                    matseq(psh, [(Wnh[:, k, col : col + 128], rhs)
                                 for k, rhs in enumerate(nh_ks)])
                    # rhn = (psh + bnh) * r ; npre = (psx + bnx) + rhn ; n = tanh(npre)
                    rhn = act.tile([128, nb], f32, tag="rhn")
                    nc.vector.scalar_tensor_tensor(
                        rhn, psh, bnh[:, m : m + 1], r, op0=ALU.add, op1=ALU.mult)
                    npre = act.tile([128, nb], f32, tag="npre")
                    nc.vector.scalar_tensor_tensor(
                        npre, psx, bnx[:, m : m + 1], rhn, op0=ALU.add, op1=ALU.add)
                    n_t = act.tile([128, nb], bf, tag="n_t")
                    nc.scalar.activation(n_t, npre, AF.Tanh)
                    # h' = n + z*(h - n)
                    d = act.tile([128, nb], bf, tag="d")
                    nc.vector.tensor_sub(d, h_al[m], n_t)
                    zd = act.tile([128, nb], bf, tag="zd")
                    nc.vector.tensor_mul(zd, z, d)
                    out_ap = out_chunk(m)
                    p = out_ap.shape[0]
                    nc.vector.tensor_add(out_ap, n_t[:p, :], zd[:p, :])

            def fc(in_ks, Wt, bias, mc, kind, out_tag):
                outs = io.tile([128, mc, nb], bf, tag=out_tag)
                for m in range(mc):
                    ps = psum.tile([128, nb], f32, tag="ps_fc")
                    matseq(ps, [(Wt[:, k, m * 128 : (m + 1) * 128], rhs)
                                for k, rhs in enumerate(in_ks)])
                    if kind == "relu":
                        nc.vector.tensor_scalar(
                            outs[:, m, :], ps, bias[:, m : m + 1], 0.0,
                            op0=ALU.add, op1=ALU.max)
                    else:
                        nc.scalar.activation(outs[:, m, :], ps, AF.Sigmoid,
                                             bias=bias[:, m : m + 1])
                return outs

            for t in range(n_tiles):
                sl = ts(t, nb)
                xh, h1s, h2s, h2p = tile0_inputs if t == 0 else load_inputs(t)

                # GRU1 -> g1: chunks 0..2 in g1a; chunk 3 (16 rows) lands in
                # partitions 0..15 of p2c3, whose partitions 16..127 get h2[0:112]
                # by DMA -- p2c3 is chunk 3 of GRU2's [g1|h2] zr operand.
                g1a = io.tile([128, 3, nb], bf, tag="g1a")
                p2c3 = io.tile([128, nb], bf, tag="p2c3")
                nc.sync.dma_start(out=p2c3[16:128, :], in_=h2_s0[:, sl])

                def g1_out(m):
                    return g1a[:, m, :] if m < 3 else p2c3[0:16, :]

                xh_ks = [xh[:, c, :] for c in range(6)]
                h1_ks = [h1s[:, c, :] for c in range(HC)]
                gru(xh_ks, W["Wzr1"], xh_ks[:3], W["Wn1x"], h1_ks, W["Wn1h"], h1_ks,
                    BT["bz1"], BT["br1"], BT["bnx1"], BT["bnh1"], g1_out)

                g1_ks = [g1a[:, 0, :], g1a[:, 1, :], g1a[:, 2, :], p2c3[:, :]]
                zr2_ks = g1_ks + [h2p[:, c, :] for c in range(3)]
                h2_ks = [h2s[:, c, :] for c in range(HC)]
                g2 = io.tile([128, HC, nb], bf, tag="g2")
                gru(zr2_ks, W["Wzr2"], g1_ks, W["Wn2x"], h2_ks, W["Wn2h"], h2_ks,
                    BT["bz2"], BT["br2"], BT["bnx2"], BT["bnh2"],
                    lambda m: g2[:, m, :])

                g2_ks = [g2[:, c, :] for c in range(HC)]
                f2 = fc(g2_ks, W["Wfc2T"], BT["bfc2"], FFp // 128, "relu", "f2")
                f3 = fc([f2[:, c, :] for c in range(FFp // 128)],
                        W["Wfc3T"], BT["bfc3"], FFp // 128, "relu", "f3")
                o = fc([f3[:, c, :] for c in range(FFp // 128)],
                       W["Wfc4T"], BT["bfc4"], Fp // 128, "sig", "o")
                nc.sync.dma_start(out=outT_r[:, :, sl], in_=o)

    nc.compile()
    return nc


def _shard_inputs(inp, weights, biases):
    x = np.asarray(inp["x"], dtype=np.float32)
    h1 = np.asarray(inp["h1"], dtype=np.float32)
    h2 = np.asarray(inp["h2"], dtype=np.float32)

    xh1T = np.zeros((NCORES, XH1, BPC), dtype=BF16)
    h2T = np.zeros((NCORES, Hp, BPC), dtype=BF16)
    for i in range(NCORES):
        sl = slice(i * BPC, (i + 1) * BPC)
        xh1T[i, :F] = x[sl].T.astype(BF16)
        xh1T[i, F : F + H] = h1[sl].T.astype(BF16)
        h2T[i, :H] = h2[sl].T.astype(BF16)

    in_maps = []
    for i in range(NCORES):
        m = {"xh1T": xh1T[i], "h2T": h2T[i]}
        m.update(weights)
        m.update(biases)
        in_maps.append(m)
    return in_maps


def _run(inp, trace=False):
    weights, biases = prepare_weights(inp)
    nc = build_nc()
    in_maps = _shard_inputs(inp, weights, biases)
    res = run_bass_kernel_spmd(nc, in_maps, list(range(NCORES)), trace=trace)
    out = np.empty((B, F), dtype=np.float32)
    for i in range(NCORES):
        out[i * BPC : (i + 1) * BPC] = (
            np.asarray(res.results[i]["outT"][:F]).astype(np.float32).T
        )
    return out, res


def kernel(**inputs) -> np.ndarray:
    out, _ = _run(inputs, trace=False)
    return out


# revision 11
# speedup vs baseline: 1.1565x; 1.1565x over previous
"""NsNet2 single-step (fc1 + 2x GRU cell + 3x FC) Trainium2 kernel.

Strategy:
  - Pure data parallel: batch B=32768 sharded as 4096 rows per NeuronCore (8 cores).
  - Feature-major ("transposed") layout on chip: activations live as [feat, batch]
    so every matmul's moving operand is already in [K, N] form -> zero on-chip
    transposes. Host transposes inputs/outputs (free; not on HW critical path).
  - bf16 matmuls (full PE rate) with fp32 PSUM accumulation; fp32 biases fused
    into ScalarE activation (sigmoid/tanh) or VectorE tensor_scalar (relu).
  - fc1 is folded into the GRU1 input-gate weights on the host (fc1 is linear and
    f1 is consumed only by GRU1's input matmuls):  (x@Wfc1.T+b) @ Wg.T =
    x @ (Wg@Wfc1).T + (Wg@b + bg).
  - z,r gates sum their input-side and hidden-side matmuls in one PSUM, so their
    contraction operands are K-concatenated ([x|h1] resp. [g1|h2]) on the host /
    on chip, saving ceil() waste: GRU1 zr K=657->6 chunks (vs 3+4), GRU2
    K=800->7 (vs 4+4).
  - Feature dims zero-padded to multiples of 128 where needed; padding never
    increases PE chunk count and keeps matmul contraction at 128 partitions.
"""

import os
import sys

import numpy as np
import ml_dtypes

sys.path.insert(0, "/opt/trn_rl_repo")

import concourse.bacc as bacc
import concourse.bass as bass
import concourse.mybir as mybir
import concourse.tile as tile
from concourse.bass import ts
from concourse.bass_utils import run_bass_kernel_spmd

BF16 = ml_dtypes.bfloat16

B, F, H, FF = 32768, 257, 400, 600
NCORES = 8
BPC = B // NCORES          # 4096 batch rows per core
Hp, FFp, Fp = 512, 640, 384  # padded feature dims
XH1 = 769                  # [x(257) | h1(400) | pad(112)] rows; 6 zr chunks + aligned h1 view at 257
ZR2K = 896                 # [g1(400) | h2(400) | pad(96)] -> 7 chunks
NB = 512                   # matmul free-dim tile (one PSUM bank of fp32)

AF = mybir.ActivationFunctionType
ALU = mybir.AluOpType

# packed bias column layout: name -> (offset, n_chunks)
BIAS_LAYOUT = {}
_off = 0
for _n, _c in (("bz1", 4), ("br1", 4), ("bnx1", 4), ("bnh1", 4),
               ("bz2", 4), ("br2", 4), ("bnx2", 4), ("bnh2", 4),
               ("bfc2", 5), ("bfc3", 5), ("bfc4", 3)):
    BIAS_LAYOUT[_n] = (_off, _c)
    _off += _c
BIAS_COLS = _off


def _pad2(a, rows, cols):
    out = np.zeros((rows, cols), dtype=np.float64)
    out[: a.shape[0], : a.shape[1]] = a
    return out


def _bias_tile(vec, padded):
    """Pack a [padded] bias vector as [128, padded//128] fp32 (partition-major)."""
    v = np.zeros(padded, dtype=np.float64)
    v[: vec.shape[0]] = vec
    return np.ascontiguousarray(v.reshape(padded // 128, 128).T).astype(np.float32)


def prepare_weights(inp):
    f64 = {k: np.asarray(v, dtype=np.float64) for k, v in inp.items()}
    w = {}

    # fc1 fold for GRU1 input side
    Wx = {}
    bx = {}
    for name in ("z", "r", "n"):
        Wx[name] = (f64[f"Wi{name}1"] @ f64["Wfc1"]).T          # [F, H]
        bx[name] = f64[f"bi{name}1"] + f64[f"Wi{name}1"] @ f64["bfc1"]

    # GRU1 z,r: K-concat [x(257) | h1(400)] -> rows 0..656 of XH1 space,
    # M = [z pad 512 | r pad 512] = 1024
    Wzr1 = np.zeros((768, 2 * Hp), dtype=np.float64)
    for g, name in enumerate(("z", "r")):
        Wzr1[:F, g * Hp : g * Hp + H] = Wx[name]
        Wzr1[F : F + H, g * Hp : g * Hp + H] = f64[f"Wh{name}1"].T
    w["Wzr1"] = Wzr1
    # GRU1 n input side: K = x chunks of XH1 (rows 0..383; rows 257+ are h1 -> zero)
    w["Wn1x"] = _pad2(Wx["n"], Fp, Hp)
    # GRU1 n hidden side: aligned h1 (XH1 rows 257..768)
    w["Wn1h"] = _pad2(f64["Whn1"].T, Hp, Hp)

    # GRU2 z,r: K-concat [g1(400) | h2(400)] -> 800 rows -> 7 chunks
    Wzr2 = np.zeros((ZR2K, 2 * Hp), dtype=np.float64)
    for g, name in enumerate(("z", "r")):
        Wzr2[:H, g * Hp : g * Hp + H] = f64[f"Wi{name}2"].T
        Wzr2[H : 2 * H, g * Hp : g * Hp + H] = f64[f"Wh{name}2"].T
    w["Wzr2"] = Wzr2
    # GRU2 n input side: K = g1 aligned (4 chunks; chunk 3 partitions 16.. are h2 -> zero)
    w["Wn2x"] = _pad2(f64["Win2"].T, Hp, Hp)
    w["Wn2h"] = _pad2(f64["Whn2"].T, Hp, Hp)

    w["Wfc2T"] = _pad2(f64["Wfc2"].T, Hp, FFp)    # [512, 640]
    w["Wfc3T"] = _pad2(f64["Wfc3"].T, FFp, FFp)   # [640, 640]
    w["Wfc4T"] = _pad2(f64["Wfc4"].T, FFp, Fp)    # [640, 384]

    weights = {k: np.ascontiguousarray(v).astype(BF16) for k, v in w.items()}

    parts = [
        ("bz1", _bias_tile(bx["z"] + f64["bhz1"], Hp)),
        ("br1", _bias_tile(bx["r"] + f64["bhr1"], Hp)),
        ("bnx1", _bias_tile(bx["n"], Hp)),
        ("bnh1", _bias_tile(f64["bhn1"], Hp)),
        ("bz2", _bias_tile(f64["biz2"] + f64["bhz2"], Hp)),
        ("br2", _bias_tile(f64["bir2"] + f64["bhr2"], Hp)),
        ("bnx2", _bias_tile(f64["bin2"], Hp)),
        ("bnh2", _bias_tile(f64["bhn2"], Hp)),
        ("bfc2", _bias_tile(f64["bfc2"], FFp)),
        ("bfc3", _bias_tile(f64["bfc3"], FFp)),
        ("bfc4", _bias_tile(f64["bfc4"], Fp)),
    ]
    biases = {"biasT": np.concatenate([p[1] for p in parts], axis=1)}
    return weights, biases


def build_nc(nbt=BPC, nb=NB):
    """Build the per-core Bass program. nbt = per-core batch, nb = free-dim tile."""
    nc = bacc.Bacc("TRN2", target_bir_lowering=False, debug=False)
    bf = mybir.dt.bfloat16
    f32 = mybir.dt.float32

    # xh1T rows: 0..256 = x.T, 257..656 = h1.T, 657..768 = zeros.
    # zr view = rows 0..767 (6 chunks); aligned-h1 view = rows 257..768 (4 chunks).
    xh1T = nc.declare_dram_parameter("xh1T", [XH1, nbt], bf, isOutput=False)
    # h2T rows 0..399 = h2.T, 400..511 zeros. Aligned view = 4 chunks; shifted
    # views for the [g1|h2] zr operand: rows 0..112 / 112..496.
    h2T = nc.declare_dram_parameter("h2T", [Hp, nbt], bf, isOutput=False)
    wd = {}
    for name, k, m in (
        ("Wzr1", 768, 2 * Hp), ("Wn1x", Fp, Hp), ("Wn1h", Hp, Hp),
        ("Wzr2", ZR2K, 2 * Hp), ("Wn2x", Hp, Hp), ("Wn2h", Hp, Hp),
        ("Wfc2T", Hp, FFp), ("Wfc3T", FFp, FFp), ("Wfc4T", FFp, Fp),
    ):
        wd[name] = nc.declare_dram_parameter(name, [k, m], bf, isOutput=False)
    biasT_d = nc.declare_dram_parameter("biasT", [128, BIAS_COLS], f32, isOutput=False)
    outT = nc.declare_dram_parameter("outT", [Fp, nbt], bf, isOutput=True)

    n_tiles = nbt // nb
    HC = Hp // 128  # 4 M-chunks per gate

    with tile.TileContext(nc) as tc:
        with (
            tc.tile_pool(name="wpool", bufs=1) as wpool,
            tc.tile_pool(name="bpool", bufs=1) as bpool,
            tc.tile_pool(name="io", bufs=2) as io,
            tc.tile_pool(name="inp", bufs=3) as inp,
            tc.tile_pool(name="act", bufs=3) as act,
            tc.tile_pool(name="psum", bufs=2, space="PSUM") as psum,
        ):
            # ACT-table warmup: first ScalarE transcendental carries the
            # ACT_TABLE_LOAD pseudo-inst; keep it off the critical chain.
            warm = bpool.tile([128, 1], f32, tag="warm")
            nc.vector.memset(warm, 0.0)
            nc.scalar.activation(warm, warm, AF.Sigmoid)

            W, BT = {}, {}

            def load_w(name):
                dram = wd[name]
                k, m = dram.shape
                t = wpool.tile([128, k // 128, m], bf, tag=name)
                r = dram.rearrange("(c p) m -> p c m", p=128)
                for c in range(k // 128):
                    nc.scalar.dma_start(out=t[:, c, :], in_=r[:, c, :])
                W[name] = t

            biasT = bpool.tile([128, BIAS_COLS], f32, tag="biasT")
            nc.scalar.dma_start(out=biasT, in_=biasT_d[:, :])
            for _n, (_o, _c) in BIAS_LAYOUT.items():
                BT[_n] = biasT[:, _o : _o + _c]

            xh_zr = xh1T[0:768, :].rearrange("(c p) n -> p c n", p=128)
            h1_al = xh1T[257 : 257 + Hp, :].rearrange("(c p) n -> p c n", p=128)
            h2_al = h2T.rearrange("(c p) n -> p c n", p=128)
            h2_s0 = h2T[0:112, :]                     # -> partitions 16..127 of zr2 chunk 3
            h2_s1 = h2T[112:496, :].rearrange("(c p) n -> p c n", p=128)
            outT_r = outT.rearrange("(c p) n -> p c n", p=128)

            def load_inputs(t):
                sl = ts(t, nb)
                xh = inp.tile([128, 6, nb], bf, tag="xh")      # zr1 K operand
                nc.sync.dma_start(out=xh, in_=xh_zr[:, :, sl])
                h1s = inp.tile([128, HC, nb], bf, tag="h1s")   # aligned h1
                nc.sync.dma_start(out=h1s, in_=h1_al[:, :, sl])
                h2s = inp.tile([128, HC, nb], bf, tag="h2s")   # aligned h2
                nc.sync.dma_start(out=h2s, in_=h2_al[:, :, sl])
                h2p = inp.tile([128, 3, nb], bf, tag="h2p")    # zr2 chunks 4..6
                nc.sync.dma_start(out=h2p, in_=h2_s1[:, :, sl])
                return xh, h1s, h2s, h2p

            tile0_inputs = load_inputs(0)

            for name in ("Wzr1", "Wn1x", "Wn1h", "Wzr2", "Wn2x", "Wn2h",
                         "Wfc2T", "Wfc3T", "Wfc4T"):
                load_w(name)

            def matseq(ps, pairs):
                n = len(pairs)
                for i, (lhsT, rhs) in enumerate(pairs):
                    nc.tensor.matmul(ps, lhsT, rhs, start=(i == 0), stop=(i == n - 1))

            def gru(zr_ks, Wzr, nx_ks, Wnx, nh_ks, Wnh, h_al,
                    bz, br, bnx, bnh, out_chunk):
                """One GRU step. zr_ks: K-chunk APs for the z,r concat operand;
                nx_ks / nh_ks: K-chunks for the n gate's two sides; h_al: aligned
                hidden chunks (blend). out_chunk(m) -> output AP for chunk m."""
                for m in range(HC):
                    zr = []
                    for g, bias in ((0, bz), (1, br)):
                        col = g * Hp + m * 128
                        ps = psum.tile([128, nb], f32, tag="ps_zr")
                        matseq(ps, [(Wzr[:, k, col : col + 128], rhs)
                                    for k, rhs in enumerate(zr_ks)])
                        gate = act.tile([128, nb], bf, tag="gate")
                        nc.scalar.activation(gate, ps, AF.Sigmoid, bias=bias[:, m : m + 1])
                        zr.append(gate)
                    z, r = zr
                    col = m * 128
                    psx = psum.tile([128, nb], f32, tag="ps_nx")
                    matseq(psx, [(Wnx[:, k, col : col + 128], rhs)
                                 for k, rhs in enumerate(nx_ks)])
                    psh = psum.tile([128, nb], f32, tag="ps_nh")
                    matseq(psh, [(Wnh[:, k, col : col + 128], rhs)
                                 for k, rhs in enumerate(nh_ks)])
                    # rhn = (psh + bnh) * r ; npre = (psx + bnx) + rhn ; n = tanh(npre)
                    rhn = act.tile([128, nb], f32, tag="rhn")
                    nc.vector.scalar_tensor_tensor(
                        rhn, psh, bnh[:, m : m + 1], r, op0=ALU.add, op1=ALU.mult)
                    npre = act.tile([128, nb], f32, tag="npre")
                    nc.vector.scalar_tensor_tensor(
                        npre, psx, bnx[:, m : m + 1], rhn, op0=ALU.add, op1=ALU.add)
                    n_t = act.tile([128, nb], bf, tag="n_t")
                    nc.scalar.activation(n_t, npre, AF.Tanh)
                    # h' = n + z*(h - n)
                    d = act.tile([128, nb], bf, tag="d")
                    nc.vector.tensor_sub(d, h_al[m], n_t)
                    zd = act.tile([128, nb], bf, tag="zd")
                    nc.vector.tensor_mul(zd, z, d)
                    out_ap = out_chunk(m)
                    p = out_ap.shape[0]
                    nc.vector.tensor_add(out_ap, n_t[:p, :], zd[:p, :])

            def fc(in_ks, Wt, bias, mc, kind, out_tag):
                outs = io.tile([128, mc, nb], bf, tag=out_tag)
                for m in range(mc):
                    ps = psum.tile([128, nb], f32, tag="ps_fc")
                    matseq(ps, [(Wt[:, k, m * 128 : (m + 1) * 128], rhs)
                                for k, rhs in enumerate(in_ks)])
                    if kind == "relu":
                        nc.vector.tensor_scalar(
                            outs[:, m, :], ps, bias[:, m : m + 1], 0.0,
                            op0=ALU.add, op1=ALU.max)
                    else:
                        nc.scalar.activation(outs[:, m, :], ps, AF.Sigmoid,
                                             bias=bias[:, m : m + 1])
                return outs

            for t in range(n_tiles):
                sl = ts(t, nb)
                xh, h1s, h2s, h2p = tile0_inputs if t == 0 else load_inputs(t)

                # GRU1 -> g1: chunks 0..2 in g1a; chunk 3 (16 rows) lands in
                # partitions 0..15 of p2c3, whose partitions 16..127 get h2[0:112]
                # by DMA -- p2c3 is chunk 3 of GRU2's [g1|h2] zr operand.
                g1a = io.tile([128, 3, nb], bf, tag="g1a")
                p2c3 = io.tile([128, nb], bf, tag="p2c3")
                nc.sync.dma_start(out=p2c3[16:128, :], in_=h2_s0[:, sl])

                def g1_out(m):
                    return g1a[:, m, :] if m < 3 else p2c3[0:16, :]

                xh_ks = [xh[:, c, :] for c in range(6)]
                h1_ks = [h1s[:, c, :] for c in range(HC)]
                gru(xh_ks, W["Wzr1"], xh_ks[:3], W["Wn1x"], h1_ks, W["Wn1h"], h1_ks,
                    BT["bz1"], BT["br1"], BT["bnx1"], BT["bnh1"], g1_out)

                g1_ks = [g1a[:, 0, :], g1a[:, 1, :], g1a[:, 2, :], p2c3[:, :]]
                zr2_ks = g1_ks + [h2p[:, c, :] for c in range(3)]
                h2_ks = [h2s[:, c, :] for c in range(HC)]
                g2 = io.tile([128, HC, nb], bf, tag="g2")
                gru(zr2_ks, W["Wzr2"], g1_ks, W["Wn2x"], h2_ks, W["Wn2h"], h2_ks,
                    BT["bz2"], BT["br2"], BT["bnx2"], BT["bnh2"],
                    lambda m: g2[:, m, :])

                g2_ks = [g2[:, c, :] for c in range(HC)]
                f2 = fc(g2_ks, W["Wfc2T"], BT["bfc2"], FFp // 128, "relu", "f2")
                f3 = fc([f2[:, c, :] for c in range(FFp // 128)],
                        W["Wfc3T"], BT["bfc3"], FFp // 128, "relu", "f3")
                o = fc([f3[:, c, :] for c in range(FFp // 128)],
                       W["Wfc4T"], BT["bfc4"], Fp // 128, "sig", "o")
                nc.sync.dma_start(out=outT_r[:, :, sl], in_=o)

    nc.compile()
    return nc


def _shard_inputs(inp, weights, biases):
    x = np.asarray(inp["x"], dtype=np.float32)
    h1 = np.asarray(inp["h1"], dtype=np.float32)
    h2 = np.asarray(inp["h2"], dtype=np.float32)

    xh1T = np.zeros((NCORES, XH1, BPC), dtype=BF16)
    h2T = np.zeros((NCORES, Hp, BPC), dtype=BF16)
    for i in range(NCORES):
        sl = slice(i * BPC, (i + 1) * BPC)
        xh1T[i, :F] = x[sl].T.astype(BF16)
        xh1T[i, F : F + H] = h1[sl].T.astype(BF16)
        h2T[i, :H] = h2[sl].T.astype(BF16)

    in_maps = []
    for i in range(NCORES):
        m = {"xh1T": xh1T[i], "h2T": h2T[i]}
        m.update(weights)
        m.update(biases)
        in_maps.append(m)
    return in_maps


def _run(inp, trace=False):
    weights, biases = prepare_weights(inp)
    nc = build_nc()
    in_maps = _shard_inputs(inp, weights, biases)
    res = run_bass_kernel_spmd(nc, in_maps, list(range(NCORES)), trace=trace)
    out = np.empty((B, F), dtype=np.float32)
    for i in range(NCORES):
        out[i * BPC : (i + 1) * BPC] = (
            np.asarray(res.results[i]["outT"][:F]).astype(np.float32).T
        )
    return out, res


def kernel(**inputs) -> np.ndarray:
    out, _ = _run(inputs, trace=False)
    return out


# revision 15
# speedup vs baseline: 1.1918x; 1.0305x over previous
"""NsNet2 single-step (fc1 + 2x GRU cell + 3x FC) Trainium2 kernel.

Strategy:
  - Pure data parallel: batch B=32768 sharded as 4096 rows per NeuronCore (8 cores).
  - Feature-major ("transposed") layout on chip: activations live as [feat, batch]
    so every matmul's moving operand is already in [K, N] form -> zero on-chip
    transposes. Host transposes inputs/outputs (free; not on HW critical path).
  - bf16 matmuls (full PE rate) with fp32 PSUM accumulation; fp32 biases fused
    into ScalarE activation (sigmoid/tanh) or VectorE tensor_scalar (relu).
  - fc1 is folded into the GRU1 input-gate weights on the host (fc1 is linear and
    f1 is consumed only by GRU1's input matmuls):  (x@Wfc1.T+b) @ Wg.T =
    x @ (Wg@Wfc1).T + (Wg@b + bg).
  - z,r gates sum their input-side and hidden-side matmuls in one PSUM, so their
    contraction operands are K-concatenated ([x|h1] resp. [g1|h2]) on the host /
    on chip, saving ceil() waste: GRU1 zr K=657->6 chunks (vs 3+4), GRU2
    K=800->7 (vs 4+4).
  - Feature dims zero-padded to multiples of 128 where needed; padding never
    increases PE chunk count and keeps matmul contraction at 128 partitions.
"""

import os
import sys

import numpy as np
import ml_dtypes

sys.path.insert(0, "/opt/trn_rl_repo")

import concourse.bacc as bacc
import concourse.bass as bass
import concourse.mybir as mybir
import concourse.tile as tile
from concourse.bass import ts
from concourse.bass_utils import run_bass_kernel_spmd

BF16 = ml_dtypes.bfloat16

B, F, H, FF = 32768, 257, 400, 600
NCORES = 8
BPC = B // NCORES          # 4096 batch rows per core
Hp, FFp, Fp = 512, 640, 384  # padded feature dims
XH1 = 769                  # [x(257) | h1(400) | pad(112)] rows; 6 zr chunks + aligned h1 view at 257
ZR2K = 896                 # [g1(400) | h2(400) | pad(96)] -> 7 chunks
ZRM = 800                  # contiguous [z(400) | r(400)] output cols -> 7 M chunks
ZRC = 7
NB = 512                   # matmul free-dim tile (one PSUM bank of fp32)

AF = mybir.ActivationFunctionType
ALU = mybir.AluOpType

# packed bias column layout: name -> (offset, n_chunks)
BIAS_LAYOUT = {}
_off = 0
for _n, _c in (("bzr1", 7), ("bnx1", 4), ("bnh1", 4),
               ("bzr2", 7), ("bnx2", 4), ("bnh2", 4),
               ("bfc2", 5), ("bfc3", 5), ("bfc4", 3)):
    BIAS_LAYOUT[_n] = (_off, _c)
    _off += _c
BIAS_COLS = _off


def _pad2(a, rows, cols):
    out = np.zeros((rows, cols), dtype=np.float64)
    out[: a.shape[0], : a.shape[1]] = a
    return out


def _bias_tile(vec, padded):
    """Pack a [padded] bias vector as [128, padded//128] fp32 (partition-major)."""
    v = np.zeros(padded, dtype=np.float64)
    v[: vec.shape[0]] = vec
    return np.ascontiguousarray(v.reshape(padded // 128, 128).T).astype(np.float32)


def prepare_weights(inp):
    f64 = {k: np.asarray(v, dtype=np.float64) for k, v in inp.items()}
    w = {}

    # fc1 fold for GRU1 input side
    Wx = {}
    bx = {}
    for name in ("z", "r", "n"):
        Wx[name] = (f64[f"Wi{name}1"] @ f64["Wfc1"]).T          # [F, H]
        bx[name] = f64[f"bi{name}1"] + f64[f"Wi{name}1"] @ f64["bfc1"]

    # GRU1 z,r: K-concat [x(257) | h1(400)] -> rows 0..656 of XH1 space,
    # M = contiguous [z(400) | r(400)] = 800 -> 7 chunks; r is lane-realigned
    # on chip by a small SBUF->SBUF DMA after the sigmoid.
    Wzr1 = np.zeros((768, ZRM), dtype=np.float64)
    for g, name in enumerate(("z", "r")):
        Wzr1[:F, g * H : g * H + H] = Wx[name]
        Wzr1[F : F + H, g * H : g * H + H] = f64[f"Wh{name}1"].T
    w["Wzr1"] = Wzr1
    # GRU1 n input side: K = x chunks of XH1 (rows 0..383; rows 257+ are h1 -> zero)
    w["Wn1x"] = _pad2(Wx["n"], Fp, Hp)
    # GRU1 n hidden side: aligned h1 (XH1 rows 257..768)
    w["Wn1h"] = _pad2(f64["Whn1"].T, Hp, Hp)

    # GRU2 z,r: K-concat [g1(400) | h2(400)] -> 800 rows -> 7 chunks
    Wzr2 = np.zeros((ZR2K, ZRM), dtype=np.float64)
    for g, name in enumerate(("z", "r")):
        Wzr2[:H, g * H : g * H + H] = f64[f"Wi{name}2"].T
        Wzr2[H : 2 * H, g * H : g * H + H] = f64[f"Wh{name}2"].T
    w["Wzr2"] = Wzr2
    # GRU2 n input side: K = g1 aligned (4 chunks; chunk 3 partitions 16.. are h2 -> zero)
    w["Wn2x"] = _pad2(f64["Win2"].T, Hp, Hp)
    w["Wn2h"] = _pad2(f64["Whn2"].T, Hp, Hp)

    w["Wfc2T"] = _pad2(f64["Wfc2"].T, Hp, FFp)    # [512, 640]
    w["Wfc3T"] = _pad2(f64["Wfc3"].T, FFp, FFp)   # [640, 640]
    w["Wfc4T"] = _pad2(f64["Wfc4"].T, FFp, Fp)    # [640, 384]

    weights = {k: np.ascontiguousarray(v).astype(BF16) for k, v in w.items()}

    parts = [
        ("bzr1", _bias_tile(np.concatenate([bx["z"] + f64["bhz1"],
                                            bx["r"] + f64["bhr1"]]), 896)),
        ("bnx1", _bias_tile(bx["n"], Hp)),
        ("bnh1", _bias_tile(f64["bhn1"], Hp)),
        ("bzr2", _bias_tile(np.concatenate([f64["biz2"] + f64["bhz2"],
                                            f64["bir2"] + f64["bhr2"]]), 896)),
        ("bnx2", _bias_tile(f64["bin2"], Hp)),
        ("bnh2", _bias_tile(f64["bhn2"], Hp)),
        ("bfc2", _bias_tile(f64["bfc2"], FFp)),
        ("bfc3", _bias_tile(f64["bfc3"], FFp)),
        ("bfc4", _bias_tile(f64["bfc4"], Fp)),
    ]
    biases = {"biasT": np.concatenate([p[1] for p in parts], axis=1)}
    return weights, biases


def build_nc(nbt=BPC, nb=NB):
    """Build the per-core Bass program. nbt = per-core batch, nb = free-dim tile."""
    nc = bacc.Bacc("TRN2", target_bir_lowering=False, debug=False)
    bf = mybir.dt.bfloat16
    f32 = mybir.dt.float32

    # xh1T rows: 0..256 = x.T, 257..656 = h1.T, 657..768 = zeros.
    # zr view = rows 0..767 (6 chunks); aligned-h1 view = rows 257..768 (4 chunks).
    xh1T = nc.declare_dram_parameter("xh1T", [XH1, nbt], bf, isOutput=False)
    # h2T rows 0..399 = h2.T, 400..511 zeros. Aligned view = 4 chunks; shifted
    # views for the [g1|h2] zr operand: rows 0..112 / 112..496.
    h2T = nc.declare_dram_parameter("h2T", [Hp, nbt], bf, isOutput=False)
    wd = {}
    for name, k, m in (
        ("Wzr1", 768, ZRM), ("Wn1x", Fp, Hp), ("Wn1h", Hp, Hp),
        ("Wzr2", ZR2K, ZRM), ("Wn2x", Hp, Hp), ("Wn2h", Hp, Hp),
        ("Wfc2T", Hp, FFp), ("Wfc3T", FFp, FFp), ("Wfc4T", FFp, Fp),
    ):
        wd[name] = nc.declare_dram_parameter(name, [k, m], bf, isOutput=False)
    biasT_d = nc.declare_dram_parameter("biasT", [128, BIAS_COLS], f32, isOutput=False)
    outT = nc.declare_dram_parameter("outT", [Fp, nbt], bf, isOutput=True)

    n_tiles = nbt // nb
    HC = Hp // 128  # 4 M-chunks per gate

    with tile.TileContext(nc) as tc:
        with (
            tc.tile_pool(name="wpool", bufs=1) as wpool,
            tc.tile_pool(name="bpool", bufs=1) as bpool,
            tc.tile_pool(name="io", bufs=2) as io,
            tc.tile_pool(name="inp", bufs=3) as inp,
            tc.tile_pool(name="act", bufs=3) as act,
            tc.tile_pool(name="psum", bufs=2, space="PSUM") as psum,
        ):
            # ACT-table warmup: first ScalarE transcendental carries the
            # ACT_TABLE_LOAD pseudo-inst; keep it off the critical chain.
            warm = bpool.tile([128, 1], f32, tag="warm")
            nc.vector.memset(warm, 0.0)
            nc.scalar.activation(warm, warm, AF.Sigmoid)

            W, BT = {}, {}

            def load_w(name, eng=None):
                dram = wd[name]
                k, m = dram.shape
                t = wpool.tile([128, k // 128, m], bf, tag=name)
                r = dram.rearrange("(c p) m -> p c m", p=128)
                for c in range(k // 128):
                    (eng or nc.sync).dma_start(out=t[:, c, :], in_=r[:, c, :])
                W[name] = t

            def load_bias():
                biasT = bpool.tile([128, BIAS_COLS], f32, tag="biasT")
                nc.sync.dma_start(out=biasT, in_=biasT_d[:, :])
                for _n, (_o, _c) in BIAS_LAYOUT.items():
                    BT[_n] = biasT[:, _o : _o + _c]

            xh_zr = xh1T[0:768, :].rearrange("(c p) n -> p c n", p=128)
            h1_al = xh1T[257 : 257 + Hp, :].rearrange("(c p) n -> p c n", p=128)
            h2_al = h2T.rearrange("(c p) n -> p c n", p=128)
            h2_s0 = h2T[0:112, :]                     # -> partitions 16..127 of zr2 chunk 3
            h2_s1 = h2T[112:496, :].rearrange("(c p) n -> p c n", p=128)
            outT_r = outT.rearrange("(c p) n -> p c n", p=128)

            def load_inputs(t):
                sl = ts(t, nb)
                xh = inp.tile([128, 6, nb], bf, tag="xh")      # zr1 K operand
                nc.sync.dma_start(out=xh, in_=xh_zr[:, :, sl])
                h1s = inp.tile([128, HC, nb], bf, tag="h1s")   # aligned h1
                nc.sync.dma_start(out=h1s, in_=h1_al[:, :, sl])
                h2s = inp.tile([128, HC, nb], bf, tag="h2s")   # aligned h2
                nc.sync.dma_start(out=h2s, in_=h2_al[:, :, sl])
                h2p = inp.tile([128, 3, nb], bf, tag="h2p")    # zr2 chunks 4..6
                nc.sync.dma_start(out=h2p, in_=h2_s1[:, :, sl])
                return xh, h1s, h2s, h2p

            tile0_inputs = load_inputs(0)

            # GRU1 weights share the sync ring with the input tiles; everything
            # needed later streams on the otherwise-idle PE ring in parallel.
            load_w("Wzr1")
            load_bias()
            for name in ("Wn1x", "Wn1h"):
                load_w(name)
            for name in ("Wzr2", "Wn2x", "Wn2h", "Wfc2T", "Wfc3T", "Wfc4T"):
                load_w(name, eng=nc.scalar)

            def matseq(ps, pairs):
                n = len(pairs)
                for i, (lhsT, rhs) in enumerate(pairs):
                    nc.tensor.matmul(ps, lhsT, rhs, start=(i == 0), stop=(i == n - 1))

            def gru(zr_ks, Wzr, nx_ks, Wnx, nh_ks, Wnh, h_al,
                    bzr, bnx, bnh, out_chunk):
                """One GRU step. zr_ks: K-chunk APs for the contiguous [z|r]
                M=800 matmul; the r half is lane-realigned (shift 16) into r_al
                via SBUF->SBUF DMA. out_chunk(m) -> output AP for chunk m."""
                # z,r preactivations: 7 contiguous M chunks (chunk 6 is 32 wide)
                zro = act.tile([128, ZRC, nb], bf, tag="zro")
                for c in range(ZRC):
                    mw = min(128, ZRM - c * 128)
                    ps = psum.tile([128, nb], f32, tag="ps_zr")
                    matseq(ps[:mw, :], [(Wzr[:, k, c * 128 : c * 128 + mw], rhs)
                                        for k, rhs in enumerate(zr_ks)])
                    nc.scalar.activation(zro[:mw, c, :], ps[:mw, :], AF.Sigmoid,
                                         bias=bzr[:mw, c : c + 1])
                # realign r (features at concat rows 400+f) to h's lanes
                r_al = act.tile([128, HC, nb], bf, tag="r_al")
                for m in range(3):
                    nc.scalar.dma_start(out=r_al[0:112, m, :], in_=zro[16:128, 3 + m, :])
                    nc.scalar.dma_start(out=r_al[112:128, m, :], in_=zro[0:16, 4 + m, :])
                nc.scalar.dma_start(out=r_al[0:16, 3, :], in_=zro[16:32, 6, :])

                for m in range(HC):
                    pz = 128 if m < 3 else 16   # valid rows of this chunk
                    col = m * 128
                    psx = psum.tile([128, nb], f32, tag="ps_nx")
                    matseq(psx, [(Wnx[:, k, col : col + 128], rhs)
                                 for k, rhs in enumerate(nx_ks)])
                    psh = psum.tile([128, nb], f32, tag="ps_nh")
                    matseq(psh, [(Wnh[:, k, col : col + 128], rhs)
                                 for k, rhs in enumerate(nh_ks)])
                    # rhn = (psh + bnh) * r ; npre = (psx + bnx) + rhn ; n = tanh(npre)
                    rhn = act.tile([128, nb], f32, tag="rhn")
                    nc.vector.scalar_tensor_tensor(
                        rhn[:pz, :], psh[:pz, :], bnh[:pz, m : m + 1],
                        r_al[:pz, m, :], op0=ALU.add, op1=ALU.mult)
                    npre = act.tile([128, nb], f32, tag="npre")
                    nc.vector.scalar_tensor_tensor(
                        npre[:pz, :], psx[:pz, :], bnx[:pz, m : m + 1],
                        rhn[:pz, :], op0=ALU.add, op1=ALU.add)
                    n_t = act.tile([128, nb], bf, tag="n_t")
                    nc.scalar.activation(n_t[:pz, :], npre[:pz, :], AF.Tanh)
                    # h' = n + z*(h - n);  z chunk m lives in zro (contig layout)
                    z_ap = zro[:pz, m, :] if m < 3 else zro[0:16, 3, :]
                    d = act.tile([128, nb], bf, tag="d")
                    nc.vector.tensor_sub(d[:pz, :], h_al[m][:pz, :], n_t[:pz, :])
                    zd = act.tile([128, nb], bf, tag="zd")
                    nc.vector.tensor_mul(zd[:pz, :], z_ap, d[:pz, :])
                    out_ap = out_chunk(m)
                    p = min(out_ap.shape[0], pz)
                    nc.vector.tensor_add(out_ap[:p, :] if out_ap.shape[0] > p else out_ap,
                                         n_t[:p, :], zd[:p, :])

            def fc(in_ks, Wt, bias, mc, kind, out_tag):
                outs = io.tile([128, mc, nb], bf, tag=out_tag)
                for m in range(mc):
                    ps = psum.tile([128, nb], f32, tag="ps_fc")
                    matseq(ps, [(Wt[:, k, m * 128 : (m + 1) * 128], rhs)
                                for k, rhs in enumerate(in_ks)])
                    if kind == "relu":
                        nc.vector.tensor_scalar(
                            outs[:, m, :], ps, bias[:, m : m + 1], 0.0,
                            op0=ALU.add, op1=ALU.max)
                    else:
                        nc.scalar.activation(outs[:, m, :], ps, AF.Sigmoid,
                                             bias=bias[:, m : m + 1])
                return outs

            for t in range(n_tiles):
                sl = ts(t, nb)
                xh, h1s, h2s, h2p = tile0_inputs if t == 0 else load_inputs(t)

                # GRU1 -> g1: chunks 0..2 in g1a; chunk 3 (16 rows) lands in
                # partitions 0..15 of p2c3, whose partitions 16..127 get h2[0:112]
                # by DMA -- p2c3 is chunk 3 of GRU2's [g1|h2] zr operand.
                g1a = io.tile([128, 3, nb], bf, tag="g1a")
                p2c3 = io.tile([128, nb], bf, tag="p2c3")
                nc.sync.dma_start(out=p2c3[16:128, :], in_=h2_s0[:, sl])

                def g1_out(m):
                    return g1a[:, m, :] if m < 3 else p2c3[0:16, :]

                xh_ks = [xh[:, c, :] for c in range(6)]
                h1_ks = [h1s[:, c, :] for c in range(HC)]
                gru(xh_ks, W["Wzr1"], xh_ks[:3], W["Wn1x"], h1_ks, W["Wn1h"], h1_ks,
                    BT["bzr1"], BT["bnx1"], BT["bnh1"], g1_out)

                g1_ks = [g1a[:, 0, :], g1a[:, 1, :], g1a[:, 2, :], p2c3[:, :]]
                zr2_ks = g1_ks + [h2p[:, c, :] for c in range(3)]
                h2_ks = [h2s[:, c, :] for c in range(HC)]
                g2 = io.tile([128, HC, nb], bf, tag="g2")
                # g2 pad rows (feature >= 400 of chunk 3) must be finite for
                # fc2's zero-weight contraction: zero them once per tile.
                nc.gpsimd.memset(g2[:, 3, :], 0.0)
                gru(zr2_ks, W["Wzr2"], g1_ks, W["Wn2x"], h2_ks, W["Wn2h"], h2_ks,
                    BT["bzr2"], BT["bnx2"], BT["bnh2"],
                    lambda m: g2[:, m, :])

                g2_ks = [g2[:, c, :] for c in range(HC)]
                f2 = fc(g2_ks, W["Wfc2T"], BT["bfc2"], FFp // 128, "relu", "f2")
                f3 = fc([f2[:, c, :] for c in range(FFp // 128)],
                        W["Wfc3T"], BT["bfc3"], FFp // 128, "relu", "f3")
                o = fc([f3[:, c, :] for c in range(FFp // 128)],
                       W["Wfc4T"], BT["bfc4"], Fp // 128, "sig", "o")
                nc.sync.dma_start(out=outT_r[:, :, sl], in_=o)

    nc.compile()
    return nc


def _shard_inputs(inp, weights, biases):
    x = np.asarray(inp["x"], dtype=np.float32)
    h1 = np.asarray(inp["h1"], dtype=np.float32)
    h2 = np.asarray(inp["h2"], dtype=np.float32)

    xh1T = np.zeros((NCORES, XH1, BPC), dtype=BF16)
    h2T = np.zeros((NCORES, Hp, BPC), dtype=BF16)
    for i in range(NCORES):
        sl = slice(i * BPC, (i + 1) * BPC)
        xh1T[i, :F] = x[sl].T.astype(BF16)
        xh1T[i, F : F + H] = h1[sl].T.astype(BF16)
        h2T[i, :H] = h2[sl].T.astype(BF16)

    in_maps = []
    for i in range(NCORES):
        m = {"xh1T": xh1T[i], "h2T": h2T[i]}
        m.update(weights)
        m.update(biases)
        in_maps.append(m)
    return in_maps


def _run(inp, trace=False):
    weights, biases = prepare_weights(inp)
    nc = build_nc()
    in_maps = _shard_inputs(inp, weights, biases)
    res = run_bass_kernel_spmd(nc, in_maps, list(range(NCORES)), trace=trace)
    out = np.empty((B, F), dtype=np.float32)
    for i in range(NCORES):
        out[i * BPC : (i + 1) * BPC] = (
            np.asarray(res.results[i]["outT"][:F]).astype(np.float32).T
        )
    return out, res


def kernel(**inputs) -> np.ndarray:
    out, _ = _run(inputs, trace=False)
    return out
